# revision 15
# baseline (speedup 1.0000x reference)
"""Trainium2 Bass kernel for nn_PraxisAttention (causal linear attention).

Sharding: 8 cores = 4 batches x 2 head-groups (tensor-parallel over the 16
heads, per the sharding hint). Core c handles batch c//2 and heads
[8*(c%2), 8*(c%2)+8). Each core computes q/k/v projections for its 1024
feature columns (bf16 matmuls, fp32 accumulate), the elu(x)+1 feature map
(min(exp(x),1)+relu(x)), causal cumulative sums over the full 4096-token
sequence via DVE prefix scans, z = per-head dot(q, k_cum) via one-hot
reduction matmuls on the PE, and the row-sharded output projection, which
yields partial sums. The host adds the two partials per batch, re-adds bo,
and transposes back.

Pipelining: the per-chunk phases are emitted one chunk deep —
A(c) [k/v proj + scans], B2(c-1) [1/z + broadcast + w tiles], B1(c)
[q proj + z + w1], OUT(c-1) [output projection] — so the PE never stalls
on the DVE reciprocal / w-tile chain at chunk boundaries (which previously
triggered HAM re-throttling).

The attention_mask input is all-ones per the problem spec (a zero entry
would make the reference divide by zero), so multiplying k/v/z by it is an
identity and is skipped on device. EPS=1e-6 in the reference z denominator
is also skipped: all feature-map outputs are strictly positive so z >> EPS
(relative effect ~1e-7).

Numerics: matmul operands bf16 (fp32 PSUM accumulation); attention-core
intermediates fp32 except w1/w tiles (bf16); final partials stored fp32.
"""

import sys

sys.path.insert(0, "/opt/trn_rl_repo")

import numpy as np
import ml_dtypes

BF16 = ml_dtypes.bfloat16

# Problem constants
B, L, D = 4, 4096, 2048
H, DH = 16, 128
N_CORES = 8
HPC = 8        # heads per core
FPC = HPC * DH  # feature columns per core (1024)
CH = 512       # tokens per chunk
NCH = L // CH  # 8 chunks
KT = D // 128  # 16 k-tiles (projection contraction)
KO = FPC // 128  # 8 k-tiles (output projection contraction)
NT = D // 128  # 16 output feature tiles
XQ = 4         # xk DMA split (quarters)

_CACHE = {}


def _build_program(loop_r=None):
    """Build the per-core program. loop_r (timing only): wrap the whole body
    in a hardware For_i loop executing it loop_r times per dispatch."""
    import concourse.tile as tile
    from concourse import mybir, bacc

    fp32 = mybir.dt.float32
    bf16 = mybir.dt.bfloat16

    nc = bacc.Bacc("TRN2", target_bir_lowering=False, debug=False,
                   enable_asserts=True, num_devices=N_CORES)

    # Inputs (host pre-arranged, see kernel()):
    # xk[c][p][kk*CH+t] = x[b].T[kk*128+p, c*CH+t]
    xk_d = nc.dram_tensor("xk", [NCH, 128, KT * CH], bf16, kind="ExternalInput").ap()
    # wX[h][p][kk*128+j] = W[kk*128+p, h*128+j] (column-sharded slice)
    wq_d = nc.dram_tensor("wq", [HPC, 128, KT * 128], bf16, kind="ExternalInput").ap()
    wk_d = nc.dram_tensor("wk", [HPC, 128, KT * 128], bf16, kind="ExternalInput").ap()
    wv_d = nc.dram_tensor("wv", [HPC, 128, KT * 128], bf16, kind="ExternalInput").ap()
    # wo[n][p][hh*128+j] = Wo[rows][hh*128+p, n*128+j] (row-sharded slice)
    wo_d = nc.dram_tensor("wo", [NT, 128, KO * 128], bf16, kind="ExternalInput").ap()
    # onehot[:, h*8+m] = 1 iff m == h
    oh_d = nc.dram_tensor("onehot", [128, HPC * HPC], mybir.dt.float32r, kind="ExternalInput").ap()
    # sel[k, h*128+m] = 1 iff k == h  (broadcast row h of zinv over 128 partitions)
    sel_d = nc.dram_tensor("sel", [HPC, HPC * 128], mybir.dt.float32r, kind="ExternalInput").ap()
    # Output: partial yT[n][p][c*CH+t] = sum over this core's features
    y_d = nc.dram_tensor("yT", [NT, 128, L], fp32, kind="ExternalOutput").ap()

    with tile.TileContext(nc) as tc:
        with (
            tc.tile_pool(name="const", bufs=1) as constp,
            tc.tile_pool(name="xk", bufs=2) as xkp,
            tc.tile_pool(name="wts", bufs=6) as wtsp,
            tc.tile_pool(name="wo", bufs=16) as wop,
            tc.tile_pool(name="tmp", bufs=8) as tmpp,
            tc.tile_pool(name="p32", bufs=4) as p32p,
            tc.tile_pool(name="kcum", bufs=10) as kcump,
            tc.tile_pool(name="kvcum", bufs=10) as kvcump,
            tc.tile_pool(name="qf", bufs=4) as qfp,
            tc.tile_pool(name="w1", bufs=12) as w1p,
            tc.tile_pool(name="wtile", bufs=18) as wtp,
            tc.tile_pool(name="small", bufs=2) as smallp,
            tc.tile_pool(name="outs", bufs=4) as outp,
            tc.tile_pool(name="pp", bufs=3, space="PSUM") as pp,
            tc.tile_pool(name="pz", bufs=1, space="PSUM") as pzp,
            tc.tile_pool(name="pzb", bufs=2, space="PSUM") as pzbp,
            tc.tile_pool(name="po", bufs=2, space="PSUM") as pop,
        ):
            import contextlib
            loop_ctx = (tc.For_i(0, loop_r, 1) if loop_r
                        else contextlib.nullcontext())
            with loop_ctx:
                _body(nc, tc, mybir, xk_d, wq_d, wk_d, wv_d, wo_d, y_d,
                      oh_d, sel_d, constp,
                      xkp, wtsp, wop, tmpp, p32p, kcump, kvcump, qfp, w1p, wtp,
                      smallp, outp, pp, pzp, pzbp, pop)

    nc.compile()
    return nc


def _body(nc, tc, mybir, xk_d, wq_d, wk_d, wv_d, wo_d, y_d,
          oh_d, sel_d, constp,
          xkp, wtsp, wop, tmpp, p32p, kcump, kvcump, qfp, w1p, wtp,
          smallp, outp, pp, pzp, pzbp, pop):
    fp32 = mybir.dt.float32
    bf16 = mybir.dt.bfloat16
    f32r = mybir.dt.float32r
    AL = mybir.AluOpType
    AF = mybir.ActivationFunctionType

    # Per-chunk state carried between phases / chunks.
    kc_prev = [None] * HPC    # k-cumsum tiles of previous chunk
    kvc_prev = [None] * HPC   # kv-cumsum tiles of previous chunk
    kc_cur = [None] * HPC
    kvc_cur = [None] * HPC
    st = {}                   # per-chunk: pz tile, w1 tiles, wh tiles

    def proj_mm(ptile, wtile, xk):
        for kk in range(KT):
            nc.tensor.matmul(
                ptile[:], wtile[:, kk * 128:(kk + 1) * 128],
                xk[:, kk * CH:(kk + 1) * CH],
                start=(kk == 0), stop=(kk == KT - 1))

    def emit_A(c):
        """k/v projections + feature map + causal scans for chunk c."""
        xk = xkp.tile([128, KT * CH], bf16, tag="xk")
        qsz = (KT // XQ) * CH
        # DMA emission order matters cold: first quarter, then head-0 k
        # weights, then the rest of xk — so head 0's matmuls start ASAP.
        nc.sync.dma_start(xk[:, 0:qsz], xk_d[c, :, 0:qsz])
        wkh0 = wtsp.tile([128, KT * 128], bf16, tag="wts")
        nc.sync.dma_start(wkh0[:], wk_d[0])
        for q in range(1, XQ):
            nc.sync.dma_start(xk[:, q * qsz:(q + 1) * qsz],
                              xk_d[c, :, q * qsz:(q + 1) * qsz])
        for h in range(HPC):
            if h == 0:
                wkh = wkh0
            else:
                wkh = wtsp.tile([128, KT * 128], bf16, tag="wts")
                nc.sync.dma_start(wkh[:], wk_d[h])
            pk = pp.tile([128, CH], fp32, tag="pp")
            proj_mm(pk, wkh, xk)
            e = tmpp.tile([128, CH], fp32, tag="tmp")
            nc.scalar.activation(e[:], pk[:], AF.Exp)
            r = tmpp.tile([128, CH], fp32, tag="tmp")
            nc.scalar.activation(r[:], pk[:], AF.Relu)
            kf = tmpp.tile([128, CH], fp32, tag="tmp")
            nc.vector.scalar_tensor_tensor(
                kf[:], e[:], 1.0, r[:], AL.min, AL.add)

            wvh = wtsp.tile([128, KT * 128], bf16, tag="wts")
            nc.sync.dma_start(wvh[:], wv_d[h])
            pv = pp.tile([128, CH], fp32, tag="pp")
            proj_mm(pv, wvh, xk)
            kv = tmpp.tile([128, CH], fp32, tag="tmp")
            # kv = kf * v, reading v straight from PSUM
            nc.vector.tensor_tensor(kv[:], kf[:], pv[:], AL.mult)

            kc = kcump.tile([128, CH], fp32, tag="kcum")
            init_k = 0.0 if c == 0 else kc_prev[h][:, CH - 1:CH]
            nc.vector.tensor_tensor_scan(
                kc[:], kf[:], kf[:], init_k, AL.add, AL.bypass)
            kvc = kvcump.tile([128, CH], fp32, tag="kvcum")
            init_kv = 0.0 if c == 0 else kvc_prev[h][:, CH - 1:CH]
            nc.vector.tensor_tensor_scan(
                kvc[:], kv[:], kv[:], init_kv, AL.add, AL.bypass)
            kc_cur[h] = kc
            kvc_cur[h] = kvc
        st[(c, "xk")] = xk

    def emit_B1(c):
        """q projection + feature map + z accumulation + w1 for chunk c."""
        pz = pzp.tile([HPC, CH], fp32, tag="pz")
        xk = st.pop((c, "xk"))
        w1s = []
        for h in range(HPC):
            wqh = wtsp.tile([128, KT * 128], bf16, tag="wts")
            nc.sync.dma_start(wqh[:], wq_d[h])
            pq = pp.tile([128, CH], fp32, tag="pp")
            proj_mm(pq, wqh, xk)
            eq = tmpp.tile([128, CH], fp32, tag="tmp")
            nc.scalar.activation(eq[:], pq[:], AF.Exp)
            rq = tmpp.tile([128, CH], fp32, tag="tmp")
            nc.scalar.activation(rq[:], pq[:], AF.Relu)
            qf = qfp.tile([128, CH], fp32, tag="qf")
            nc.vector.scalar_tensor_tensor(
                qf[:], eq[:], 1.0, rq[:], AL.min, AL.add)
            p = p32p.tile([128, CH], f32r, tag="p32r")
            with nc.allow_low_precision(reason="f32r feeds full-rate PE z-reduce"):
                nc.vector.tensor_tensor(p[:], qf[:], kc_cur[h][:], AL.mult)
            nc.tensor.matmul(
                pz[:], consts["onehot"][:, h * HPC:(h + 1) * HPC], p[:],
                start=(h == 0), stop=(h == HPC - 1))
            w1 = w1p.tile([128, CH], bf16, tag="w1")
            nc.vector.tensor_tensor(w1[:], qf[:], kvc_cur[h][:], AL.mult)
            w1s.append(w1)
        st[(c, "pz")] = pz
        st[(c, "w1")] = w1s

    def emit_B2(c, t0=0, t1=CH):
        """1/z + per-head broadcast + w tiles for chunk c, tokens [t0,t1)."""
        pz = st[(c, "pz")]
        w1s = st[(c, "w1")]
        tw = t1 - t0
        zinv = smallp.tile([HPC, CH], f32r, tag="zinv")
        with nc.allow_low_precision(reason="f32r feeds full-rate PE broadcast"):
            nc.vector.reciprocal(zinv[:, :tw], pz[:, t0:t1])
        whs = []
        for h in range(HPC):
            pzb = pzbp.tile([128, CH], fp32, tag="pzb")
            nc.tensor.matmul(pzb[:, :tw], consts["sel"][:, h * 128:(h + 1) * 128],
                             zinv[:, :tw], start=True, stop=True)
            wh = wtp.tile([128, CH], bf16, tag="wtile")
            nc.vector.tensor_tensor(wh[:, :tw], w1s[h][:, t0:t1], pzb[:, :tw],
                                    AL.mult)
            whs.append(wh)
        st[(c, "wh", t0)] = whs
        if t1 == CH:
            st.pop((c, "pz"))
            st.pop((c, "w1"))

    def emit_OUT(c, t0=0, t1=CH):
        """Row-sharded output projection partial for chunk c, tokens [t0,t1)."""
        whs = st.pop((c, "wh", t0))
        tw = t1 - t0
        for n in range(NT):
            won = wo_res[n]
            po = pop.tile([128, CH], fp32, tag="po")
            for hh in range(KO):
                nc.tensor.matmul(
                    po[:, :tw], won[:, hh * 128:(hh + 1) * 128],
                    whs[hh][:, :tw],
                    start=(hh == 0), stop=(hh == KO - 1))
            ot = outp.tile([128, CH], fp32, tag="outs")
            nc.scalar.copy(ot[:, :tw], po[:, :tw])
            nc.sync.dma_start(y_d[n, :, c * CH + t0:c * CH + t1], ot[:, :tw])

    wo_res = [None] * NT
    consts = {}
    for c in range(NCH):
        emit_A(c)
        if c == 0:
            # Constants + resident output-projection weights: emitted after
            # A(0) so the first chunk's xk/weight DMAs win queue priority.
            onehot = constp.tile([128, HPC * HPC], mybir.dt.float32r)
            nc.sync.dma_start(onehot[:], oh_d[:])
            sel = constp.tile([HPC, HPC * 128], mybir.dt.float32r)
            nc.sync.dma_start(sel[:], sel_d[:])
            consts["onehot"] = onehot
            consts["sel"] = sel
            for n in range(NT):
                won = wop.tile([128, KO * 128], bf16, tag="wo")
                nc.sync.dma_start(won[:], wo_d[n])
                wo_res[n] = won
        if c > 0:
            emit_B2(c - 1)
        emit_B1(c)
        if c > 0:
            emit_OUT(c - 1)
        kc_prev = list(kc_cur)
        kvc_prev = list(kvc_cur)
    emit_B2(NCH - 1)
    emit_OUT(NCH - 1)


def _get_program():
    if "nc" not in _CACHE:
        _CACHE["nc"] = _build_program()
    return _CACHE["nc"]


def _prep_inputs(x, Wq, Wk, Wv, Wo):
    """Host-side shard + rearrange + cast. Returns per-core input maps."""
    def arrange_w_cols(W, g):
        # W[:, g*FPC:(g+1)*FPC] -> [HPC, 128, KT*128]
        Ws = np.ascontiguousarray(W[:, g * FPC:(g + 1) * FPC]).astype(BF16)
        return np.ascontiguousarray(
            Ws.reshape(KT, 128, HPC, 128).transpose(2, 1, 0, 3)
        ).reshape(HPC, 128, KT * 128)

    def arrange_wo_rows(W, g):
        # W[g*FPC:(g+1)*FPC, :] -> [NT, 128, KO*128]
        Ws = np.ascontiguousarray(W[g * FPC:(g + 1) * FPC, :]).astype(BF16)
        return np.ascontiguousarray(
            Ws.reshape(KO, 128, NT, 128).transpose(2, 1, 0, 3)
        ).reshape(NT, 128, KO * 128)

    onehot = np.zeros((128, HPC * HPC), np.float32)
    for h in range(HPC):
        onehot[:, h * HPC + h] = 1.0
    sel = np.zeros((HPC, HPC * 128), np.float32)
    for h in range(HPC):
        sel[h, h * 128:(h + 1) * 128] = 1.0

    w_by_g = []
    for g in range(2):
        w_by_g.append({
            "wq": arrange_w_cols(Wq, g),
            "wk": arrange_w_cols(Wk, g),
            "wv": arrange_w_cols(Wv, g),
            "wo": arrange_wo_rows(Wo, g),
        })

    xk_by_b = []
    for b in range(B):
        xT = np.ascontiguousarray(x[b].T).astype(BF16)  # [D, L]
        xk = np.ascontiguousarray(
            xT.reshape(KT, 128, NCH, CH).transpose(2, 1, 0, 3)
        ).reshape(NCH, 128, KT * CH)
        xk_by_b.append(xk)

    in_maps = []
    for c in range(N_CORES):
        b, g = c // 2, c % 2
        m = {"xk": xk_by_b[b], "onehot": onehot, "sel": sel}
        m.update(w_by_g[g])
        in_maps.append(m)
    return in_maps


def _gather_output(results, bo):
    out = np.empty((B, L, D), np.float32)
    for b in range(B):
        yp = results[2 * b]["yT"] + results[2 * b + 1]["yT"]  # [NT,128,L]
        # yT[n, p, t] = out[t, n*128+p]
        out[b] = yp.reshape(NT * 128, L).T + bo[None, :]
    return out


def kernel(x, attention_mask, Wq, bq, Wk, bk, Wv, bv, Wo, bo, **_ignored):
    from concourse.bass_utils import run_bass_kernel_spmd

    x = np.asarray(x, np.float32)
    nc = _get_program()
    # bq/bk/bv are zero in this problem; q/k/v biases are additive constants
    # folded on host would be wrong (nonlinear feature map), so assert.
    assert not np.any(bq) and not np.any(bk) and not np.any(bv), \
        "kernel compiled for zero q/k/v biases"
    in_maps = _prep_inputs(x, np.asarray(Wq), np.asarray(Wk), np.asarray(Wv),
                           np.asarray(Wo))
    res = run_bass_kernel_spmd(nc, in_maps, list(range(N_CORES)))
    return _gather_output(res.results, np.asarray(bo, np.float32))


# revision 19
# speedup vs baseline: 1.0130x; 1.0130x over previous
"""Trainium2 Bass kernel for nn_PraxisAttention (causal linear attention).

Sharding: 8 cores = 4 batches x 2 head-groups (tensor-parallel over the 16
heads, per the sharding hint). Core c handles batch c//2 and heads
[8*(c%2), 8*(c%2)+8). Each core computes q/k/v projections for its 1024
feature columns (bf16 matmuls, fp32 accumulate), the elu(x)+1 feature map
(min(exp(x),1)+relu(x)), causal cumulative sums over the full 4096-token
sequence via DVE prefix scans, z = per-head dot(q, k_cum) via one-hot
reduction matmuls on the PE, and the row-sharded output projection, which
yields partial sums. The host adds the two partials per batch, re-adds bo,
and transposes back.

Pipelining: the per-chunk phases are emitted one chunk deep —
A(c) [k/v proj + scans], B2(c-1) [1/z + broadcast + w tiles], B1(c)
[q proj + z + w1], OUT(c-1) [output projection] — so the PE never stalls
on the DVE reciprocal / w-tile chain at chunk boundaries (which previously
triggered HAM re-throttling).

The attention_mask input is all-ones per the problem spec (a zero entry
would make the reference divide by zero), so multiplying k/v/z by it is an
identity and is skipped on device. EPS=1e-6 in the reference z denominator
is also skipped: all feature-map outputs are strictly positive so z >> EPS
(relative effect ~1e-7).

Numerics: matmul operands bf16 (fp32 PSUM accumulation); attention-core
intermediates fp32 except w1/w tiles (bf16); final partials stored fp32.
"""

import sys

sys.path.insert(0, "/opt/trn_rl_repo")

import numpy as np
import ml_dtypes

BF16 = ml_dtypes.bfloat16

# Problem constants
B, L, D = 4, 4096, 2048
H, DH = 16, 128
N_CORES = 8
HPC = 8        # heads per core
FPC = HPC * DH  # feature columns per core (1024)
CH = 512       # tokens per chunk
NCH = L // CH  # 8 chunks
KT = D // 128  # 16 k-tiles (projection contraction)
KO = FPC // 128  # 8 k-tiles (output projection contraction)
NT = D // 128  # 16 output feature tiles
XQ = 4         # xk DMA split (quarters)

_CACHE = {}


def _build_program(loop_r=None):
    """Build the per-core program. loop_r (timing only): wrap the whole body
    in a hardware For_i loop executing it loop_r times per dispatch."""
    import concourse.tile as tile
    from concourse import mybir, bacc

    fp32 = mybir.dt.float32
    bf16 = mybir.dt.bfloat16

    nc = bacc.Bacc("TRN2", target_bir_lowering=False, debug=False,
                   enable_asserts=True, num_devices=N_CORES)

    # Inputs (host pre-arranged, see kernel()):
    # xk[c][p][kk*CH+t] = x[b].T[kk*128+p, c*CH+t]
    xk_d = nc.dram_tensor("xk", [NCH, 128, KT * CH], bf16, kind="ExternalInput").ap()
    # wX[h][p][kk*128+j] = W[kk*128+p, h*128+j] (column-sharded slice)
    wq_d = nc.dram_tensor("wq", [HPC, 128, KT * 128], bf16, kind="ExternalInput").ap()
    wk_d = nc.dram_tensor("wk", [HPC, 128, KT * 128], bf16, kind="ExternalInput").ap()
    wv_d = nc.dram_tensor("wv", [HPC, 128, KT * 128], bf16, kind="ExternalInput").ap()
    # wo[n][p][hh*128+j] = Wo[rows][hh*128+p, n*128+j] (row-sharded slice)
    wo_d = nc.dram_tensor("wo", [NT, 128, KO * 128], bf16, kind="ExternalInput").ap()
    # onehot[:, h*8+m] = 1 iff m == h
    oh_d = nc.dram_tensor("onehot", [128, HPC * 128], bf16, kind="ExternalInput").ap()
    # sel[k, h*128+m] = 1 iff k == h  (broadcast row h of zinv over 128 partitions)
    sel_d = nc.dram_tensor("sel", [HPC, HPC * 128], bf16, kind="ExternalInput").ap()
    # Output: partial yT[n][p][c*CH+t] = sum over this core's features
    y_d = nc.dram_tensor("yT", [NT, 128, L], fp32, kind="ExternalOutput").ap()

    with tile.TileContext(nc) as tc:
        with (
            tc.tile_pool(name="const", bufs=1) as constp,
            tc.tile_pool(name="xk", bufs=2) as xkp,
            tc.tile_pool(name="wts", bufs=6) as wtsp,
            tc.tile_pool(name="wo", bufs=16) as wop,
            tc.tile_pool(name="tmp", bufs=8) as tmpp,
            tc.tile_pool(name="p32", bufs=4) as p32p,
            tc.tile_pool(name="kcum", bufs=10) as kcump,
            tc.tile_pool(name="kvcum", bufs=9) as kvcump,
            tc.tile_pool(name="qf", bufs=10) as qfp,
            tc.tile_pool(name="w1", bufs=10) as w1p,
            tc.tile_pool(name="wtile", bufs=16) as wtp,
            tc.tile_pool(name="small", bufs=2) as smallp,
            tc.tile_pool(name="outs", bufs=4) as outp,
            tc.tile_pool(name="pp", bufs=3, space="PSUM") as pp,
            tc.tile_pool(name="pz", bufs=1, space="PSUM") as pzp,
            tc.tile_pool(name="pzb", bufs=2, space="PSUM") as pzbp,
            tc.tile_pool(name="po", bufs=2, space="PSUM") as pop,
        ):
            import contextlib
            loop_ctx = (tc.For_i(0, loop_r, 1) if loop_r
                        else contextlib.nullcontext())
            with loop_ctx:
                _body(nc, tc, mybir, xk_d, wq_d, wk_d, wv_d, wo_d, y_d,
                      oh_d, sel_d, constp,
                      xkp, wtsp, wop, tmpp, p32p, kcump, kvcump, qfp, w1p, wtp,
                      smallp, outp, pp, pzp, pzbp, pop)

    nc.compile()
    return nc


def _body(nc, tc, mybir, xk_d, wq_d, wk_d, wv_d, wo_d, y_d,
          oh_d, sel_d, constp,
          xkp, wtsp, wop, tmpp, p32p, kcump, kvcump, qfp, w1p, wtp,
          smallp, outp, pp, pzp, pzbp, pop):
    fp32 = mybir.dt.float32
    bf16 = mybir.dt.bfloat16
    f32r = mybir.dt.float32r
    AL = mybir.AluOpType
    AF = mybir.ActivationFunctionType

    # Per-chunk state carried between phases / chunks.
    kc_prev = [None] * HPC    # k-cumsum tiles of previous chunk
    kvc_prev = [None] * HPC   # kv-cumsum tiles of previous chunk
    kc_cur = [None] * HPC
    kvc_cur = [None] * HPC
    st = {}                   # per-chunk: pz tile, w1 tiles, wh tiles

    def proj_mm(ptile, wtile, xk):
        for kk in range(KT):
            nc.tensor.matmul(
                ptile[:], wtile[:, kk * 128:(kk + 1) * 128],
                xk[:, kk * CH:(kk + 1) * CH],
                start=(kk == 0), stop=(kk == KT - 1))

    def emit_A(c):
        """k/v projections + feature map + causal scans for chunk c."""
        xk = xkp.tile([128, KT * CH], bf16, tag="xk")
        qsz = (KT // XQ) * CH
        # DMA emission order matters cold: first quarter, then head-0 k
        # weights, then the rest of xk — so head 0's matmuls start ASAP.
        nc.sync.dma_start(xk[:, 0:qsz], xk_d[c, :, 0:qsz])
        wkh0 = wtsp.tile([128, KT * 128], bf16, tag="wts")
        nc.sync.dma_start(wkh0[:], wk_d[0])
        for q in range(1, XQ):
            nc.sync.dma_start(xk[:, q * qsz:(q + 1) * qsz],
                              xk_d[c, :, q * qsz:(q + 1) * qsz])
        for h in range(HPC):
            if h == 0:
                wkh = wkh0
            else:
                wkh = wtsp.tile([128, KT * 128], bf16, tag="wts")
                nc.sync.dma_start(wkh[:], wk_d[h])
            pk = pp.tile([128, CH], fp32, tag="pp")
            proj_mm(pk, wkh, xk)
            e = tmpp.tile([128, CH], fp32, tag="tmp")
            nc.scalar.activation(e[:], pk[:], AF.Exp)
            r = tmpp.tile([128, CH], fp32, tag="tmp")
            nc.scalar.activation(r[:], pk[:], AF.Relu)
            kf = tmpp.tile([128, CH], fp32, tag="tmp")
            nc.vector.scalar_tensor_tensor(
                kf[:], e[:], 1.0, r[:], AL.min, AL.add)

            wvh = wtsp.tile([128, KT * 128], bf16, tag="wts")
            nc.sync.dma_start(wvh[:], wv_d[h])
            pv = pp.tile([128, CH], fp32, tag="pp")
            proj_mm(pv, wvh, xk)
            kv = tmpp.tile([128, CH], fp32, tag="tmp")
            # kv = kf * v, reading v straight from PSUM
            nc.vector.tensor_tensor(kv[:], kf[:], pv[:], AL.mult)

            kc = kcump.tile([128, CH], fp32, tag="kcum")
            init_k = 0.0 if c == 0 else kc_prev[h][:, CH - 1:CH]
            nc.vector.tensor_tensor_scan(
                kc[:], kf[:], kf[:], init_k, AL.add, AL.bypass)
            kvc = kvcump.tile([128, CH], fp32, tag="kvcum")
            init_kv = 0.0 if c == 0 else kvc_prev[h][:, CH - 1:CH]
            nc.vector.tensor_tensor_scan(
                kvc[:], kv[:], kv[:], init_kv, AL.add, AL.bypass)
            kc_cur[h] = kc
            kvc_cur[h] = kvc
        st[(c, "xk")] = xk

    def emit_B1(c):
        """q projection + feature map + z accumulation + w1 for chunk c."""
        pz = pzp.tile([128, CH], fp32, tag="pz")
        xk = st.pop((c, "xk"))
        w1s = []
        for h in range(HPC):
            wqh = wtsp.tile([128, KT * 128], bf16, tag="wts")
            nc.sync.dma_start(wqh[:], wq_d[h])
            pq = pp.tile([128, CH], fp32, tag="pp")
            proj_mm(pq, wqh, xk)
            eq = tmpp.tile([128, CH], fp32, tag="tmp")
            nc.scalar.activation(eq[:], pq[:], AF.Exp)
            rq = tmpp.tile([128, CH], fp32, tag="tmp")
            nc.scalar.activation(rq[:], pq[:], AF.Relu)
            qf = qfp.tile([128, CH], fp32, tag="qf")
            nc.vector.scalar_tensor_tensor(
                qf[:], eq[:], 1.0, rq[:], AL.min, AL.add)
            p = p32p.tile([128, CH], bf16, tag="p32r")
            with nc.allow_low_precision(reason="bf16 feeds FWL-rate PE z-reduce"):
                nc.vector.tensor_tensor(p[:], qf[:], kc_cur[h][:], AL.mult)
            nc.tensor.matmul(
                pz[:], consts["onehot"][:, h * 128:(h + 1) * 128], p[:],
                start=(h == 0), stop=(h == HPC - 1))
            w1s.append((qf, kvc_cur[h]))
        st[(c, "pz")] = pz
        st[(c, "w1")] = w1s

    def emit_B2(c, t0=0, t1=CH):
        """1/z + per-head broadcast + w tiles for chunk c, tokens [t0,t1)."""
        pz = st[(c, "pz")]
        w1s = st[(c, "w1")]
        if w1s and isinstance(w1s[0], tuple):
            mats = []
            for qf_t, kvc_t in w1s:
                w1 = w1p.tile([128, CH], bf16, tag="w1")
                nc.vector.tensor_tensor(w1[:], qf_t[:], kvc_t[:], AL.mult)
                mats.append(w1)
            w1s = st[(c, "w1")] = mats
        tw = t1 - t0
        zinv = smallp.tile([HPC, CH], bf16, tag="zinv")
        with nc.allow_low_precision(reason="bf16 feeds FWL-rate PE broadcast"):
            nc.vector.reciprocal(zinv[:, :tw], pz[0:HPC, t0:t1])
        whs = []
        for h in range(HPC):
            pzb = pzbp.tile([128, CH], fp32, tag="pzb")
            nc.tensor.matmul(pzb[:, :tw], consts["sel"][:, h * 128:(h + 1) * 128],
                             zinv[:, :tw], start=True, stop=True)
            wh = wtp.tile([128, CH], bf16, tag="wtile")
            nc.vector.tensor_tensor(wh[:, :tw], w1s[h][:, t0:t1], pzb[:, :tw],
                                    AL.mult)
            whs.append(wh)
        st[(c, "wh", t0)] = whs
        if t1 == CH:
            st.pop((c, "pz"))
            st.pop((c, "w1"))

    def emit_OUT(c, t0=0, t1=CH):
        """Row-sharded output projection partial for chunk c, tokens [t0,t1)."""
        whs = st.pop((c, "wh", t0))
        tw = t1 - t0
        for n in range(NT):
            won = wo_res[n]
            po = pop.tile([128, CH], fp32, tag="po")
            for hh in range(KO):
                nc.tensor.matmul(
                    po[:, :tw], won[:, hh * 128:(hh + 1) * 128],
                    whs[hh][:, :tw],
                    start=(hh == 0), stop=(hh == KO - 1))
            ot = outp.tile([128, CH], fp32, tag="outs")
            nc.scalar.copy(ot[:, :tw], po[:, :tw])
            nc.sync.dma_start(y_d[n, :, c * CH + t0:c * CH + t1], ot[:, :tw])

    wo_res = [None] * NT
    consts = {}
    for c in range(NCH):
        emit_A(c)
        if c == 0:
            # Constants + resident output-projection weights: emitted after
            # A(0) so the first chunk's xk/weight DMAs win queue priority.
            onehot = constp.tile([128, HPC * 128], bf16)
            nc.sync.dma_start(onehot[:], oh_d[:])
            sel = constp.tile([HPC, HPC * 128], bf16)
            nc.sync.dma_start(sel[:], sel_d[:])
            consts["onehot"] = onehot
            consts["sel"] = sel
            for n in range(NT):
                won = wop.tile([128, KO * 128], bf16, tag="wo")
                nc.sync.dma_start(won[:], wo_d[n])
                wo_res[n] = won
        if c > 0:
            emit_B2(c - 1)
        emit_B1(c)
        if c > 0:
            emit_OUT(c - 1)
        kc_prev = list(kc_cur)
        kvc_prev = list(kvc_cur)
    emit_B2(NCH - 1)
    emit_OUT(NCH - 1)


def _get_program():
    if "nc" not in _CACHE:
        _CACHE["nc"] = _build_program()
    return _CACHE["nc"]


def _prep_inputs(x, Wq, Wk, Wv, Wo):
    """Host-side shard + rearrange + cast. Returns per-core input maps."""
    def arrange_w_cols(W, g):
        # W[:, g*FPC:(g+1)*FPC] -> [HPC, 128, KT*128]
        Ws = np.ascontiguousarray(W[:, g * FPC:(g + 1) * FPC]).astype(BF16)
        return np.ascontiguousarray(
            Ws.reshape(KT, 128, HPC, 128).transpose(2, 1, 0, 3)
        ).reshape(HPC, 128, KT * 128)

    def arrange_wo_rows(W, g):
        # W[g*FPC:(g+1)*FPC, :] -> [NT, 128, KO*128]
        Ws = np.ascontiguousarray(W[g * FPC:(g + 1) * FPC, :]).astype(BF16)
        return np.ascontiguousarray(
            Ws.reshape(KO, 128, NT, 128).transpose(2, 1, 0, 3)
        ).reshape(NT, 128, KO * 128)

    onehot = np.zeros((128, HPC * 128), np.float32)
    for h in range(HPC):
        onehot[:, h * 128 + h] = 1.0
    onehot = onehot.astype(BF16)
    sel = np.zeros((HPC, HPC * 128), np.float32)
    for h in range(HPC):
        sel[h, h * 128:(h + 1) * 128] = 1.0
    sel = sel.astype(BF16)

    w_by_g = []
    for g in range(2):
        w_by_g.append({
            "wq": arrange_w_cols(Wq, g),
            "wk": arrange_w_cols(Wk, g),
            "wv": arrange_w_cols(Wv, g),
            "wo": arrange_wo_rows(Wo, g),
        })

    xk_by_b = []
    for b in range(B):
        xT = np.ascontiguousarray(x[b].T).astype(BF16)  # [D, L]
        xk = np.ascontiguousarray(
            xT.reshape(KT, 128, NCH, CH).transpose(2, 1, 0, 3)
        ).reshape(NCH, 128, KT * CH)
        xk_by_b.append(xk)

    in_maps = []
    for c in range(N_CORES):
        b, g = c // 2, c % 2
        m = {"xk": xk_by_b[b], "onehot": onehot, "sel": sel}
        m.update(w_by_g[g])
        in_maps.append(m)
    return in_maps


def _gather_output(results, bo):
    out = np.empty((B, L, D), np.float32)
    for b in range(B):
        yp = results[2 * b]["yT"] + results[2 * b + 1]["yT"]  # [NT,128,L]
        # yT[n, p, t] = out[t, n*128+p]
        out[b] = yp.reshape(NT * 128, L).T + bo[None, :]
    return out


def kernel(x, attention_mask, Wq, bq, Wk, bk, Wv, bv, Wo, bo, **_ignored):
    from concourse.bass_utils import run_bass_kernel_spmd

    x = np.asarray(x, np.float32)
    nc = _get_program()
    # bq/bk/bv are zero in this problem; q/k/v biases are additive constants
    # folded on host would be wrong (nonlinear feature map), so assert.
    assert not np.any(bq) and not np.any(bk) and not np.any(bv), \
        "kernel compiled for zero q/k/v biases"
    in_maps = _prep_inputs(x, np.asarray(Wq), np.asarray(Wk), np.asarray(Wv),
                           np.asarray(Wo))
    res = run_bass_kernel_spmd(nc, in_maps, list(range(N_CORES)))
    return _gather_output(res.results, np.asarray(bo, np.float32))


# revision 22
# speedup vs baseline: 1.0139x; 1.0009x over previous
"""Trainium2 Bass kernel for nn_PraxisAttention (causal linear attention).

Sharding: 8 cores = 4 batches x 2 head-groups (tensor-parallel over the 16
heads, per the sharding hint). Core c handles batch c//2 and heads
[8*(c%2), 8*(c%2)+8). Each core computes q/k/v projections for its 1024
feature columns (bf16 matmuls, fp32 accumulate), the elu(x)+1 feature map
(min(exp(x),1)+relu(x)), causal cumulative sums over the full 4096-token
sequence via DVE prefix scans, z = per-head dot(q, k_cum) via one-hot
reduction matmuls on the PE, and the row-sharded output projection, which
yields partial sums. The host adds the two partials per batch, re-adds bo,
and transposes back.

Pipelining: the per-chunk phases are emitted one chunk deep —
A(c) [k/v proj + scans], B2(c-1) [1/z + broadcast + w tiles], B1(c)
[q proj + z + w1], OUT(c-1) [output projection] — so the PE never stalls
on the DVE reciprocal / w-tile chain at chunk boundaries (which previously
triggered HAM re-throttling).

The attention_mask input is all-ones per the problem spec (a zero entry
would make the reference divide by zero), so multiplying k/v/z by it is an
identity and is skipped on device. EPS=1e-6 in the reference z denominator
is also skipped: all feature-map outputs are strictly positive so z >> EPS
(relative effect ~1e-7).

Numerics: matmul operands bf16 (fp32 PSUM accumulation); attention-core
intermediates fp32 except w1/w tiles (bf16); final partials stored fp32.
"""

import sys

sys.path.insert(0, "/opt/trn_rl_repo")

import numpy as np
import ml_dtypes

BF16 = ml_dtypes.bfloat16

# Problem constants
B, L, D = 4, 4096, 2048
H, DH = 16, 128
N_CORES = 8
HPC = 8        # heads per core
FPC = HPC * DH  # feature columns per core (1024)
CH = 512       # tokens per chunk
NCH = L // CH  # 8 chunks
KT = D // 128  # 16 k-tiles (projection contraction)
KO = FPC // 128  # 8 k-tiles (output projection contraction)
NT = D // 128  # 16 output feature tiles
XQ = 4         # xk DMA split (quarters)

_CACHE = {}


def _build_program(loop_r=None):
    """Build the per-core program. loop_r (timing only): wrap the whole body
    in a hardware For_i loop executing it loop_r times per dispatch."""
    import concourse.tile as tile
    from concourse import mybir, bacc

    fp32 = mybir.dt.float32
    bf16 = mybir.dt.bfloat16

    nc = bacc.Bacc("TRN2", target_bir_lowering=False, debug=False,
                   enable_asserts=True, num_devices=N_CORES)

    # Inputs (host pre-arranged, see kernel()):
    # xk[c][p][kk*CH+t] = x[b].T[kk*128+p, c*CH+t]
    xk_d = nc.dram_tensor("xk", [NCH, 128, KT * CH], bf16, kind="ExternalInput").ap()
    # wX[h][p][kk*128+j] = W[kk*128+p, h*128+j] (column-sharded slice)
    wq_d = nc.dram_tensor("wq", [HPC, 128, KT * 128], bf16, kind="ExternalInput").ap()
    wk_d = nc.dram_tensor("wk", [HPC, 128, KT * 128], bf16, kind="ExternalInput").ap()
    wv_d = nc.dram_tensor("wv", [HPC, 128, KT * 128], bf16, kind="ExternalInput").ap()
    # wo[n][p][hh*128+j] = Wo[rows][hh*128+p, n*128+j] (row-sharded slice)
    wo_d = nc.dram_tensor("wo", [NT, 128, KO * 128], bf16, kind="ExternalInput").ap()
    # onehot[:, h*8+m] = 1 iff m == h
    oh_d = nc.dram_tensor("onehot", [128, HPC * 128], bf16, kind="ExternalInput").ap()
    # sel[k, h*128+m] = 1 iff k == h  (broadcast row h of zinv over 128 partitions)
    sel_d = nc.dram_tensor("sel", [HPC, HPC * 128], bf16, kind="ExternalInput").ap()
    # Output: partial yT[n][p][c*CH+t] = sum over this core's features
    y_d = nc.dram_tensor("yT", [NT, 128, L], fp32, kind="ExternalOutput").ap()

    with tile.TileContext(nc) as tc:
        with (
            tc.tile_pool(name="const", bufs=1) as constp,
            tc.tile_pool(name="xk", bufs=2) as xkp,
            tc.tile_pool(name="wts", bufs=6) as wtsp,
            tc.tile_pool(name="wo", bufs=16) as wop,
            tc.tile_pool(name="tmp", bufs=8) as tmpp,
            tc.tile_pool(name="p32", bufs=4) as p32p,
            tc.tile_pool(name="kcum", bufs=10) as kcump,
            tc.tile_pool(name="kvcum", bufs=9) as kvcump,
            tc.tile_pool(name="qf", bufs=10) as qfp,
            tc.tile_pool(name="w1", bufs=10) as w1p,
            tc.tile_pool(name="wtile", bufs=16) as wtp,
            tc.tile_pool(name="small", bufs=2) as smallp,
            tc.tile_pool(name="outs", bufs=4) as outp,
            tc.tile_pool(name="pp", bufs=3, space="PSUM") as pp,
            tc.tile_pool(name="pz", bufs=1, space="PSUM") as pzp,
            tc.tile_pool(name="pzb", bufs=2, space="PSUM") as pzbp,
            tc.tile_pool(name="po", bufs=2, space="PSUM") as pop,
        ):
            import contextlib
            loop_ctx = (tc.For_i(0, loop_r, 1) if loop_r
                        else contextlib.nullcontext())
            with loop_ctx:
                _body(nc, tc, mybir, xk_d, wq_d, wk_d, wv_d, wo_d, y_d,
                      oh_d, sel_d, constp,
                      xkp, wtsp, wop, tmpp, p32p, kcump, kvcump, qfp, w1p, wtp,
                      smallp, outp, pp, pzp, pzbp, pop)

    nc.compile()
    return nc


def _body(nc, tc, mybir, xk_d, wq_d, wk_d, wv_d, wo_d, y_d,
          oh_d, sel_d, constp,
          xkp, wtsp, wop, tmpp, p32p, kcump, kvcump, qfp, w1p, wtp,
          smallp, outp, pp, pzp, pzbp, pop):
    fp32 = mybir.dt.float32
    bf16 = mybir.dt.bfloat16
    f32r = mybir.dt.float32r
    AL = mybir.AluOpType
    AF = mybir.ActivationFunctionType

    # Per-chunk state carried between phases / chunks.
    kc_prev = [None] * HPC    # k-cumsum tiles of previous chunk
    kvc_prev = [None] * HPC   # kv-cumsum tiles of previous chunk
    kc_cur = [None] * HPC
    kvc_cur = [None] * HPC
    st = {}                   # per-chunk: pz tile, w1 tiles, wh tiles

    def proj_mm(ptile, wtile, xk):
        for kk in range(KT):
            nc.tensor.matmul(
                ptile[:], wtile[:, kk * 128:(kk + 1) * 128],
                xk[:, kk * CH:(kk + 1) * CH],
                start=(kk == 0), stop=(kk == KT - 1))

    def emit_A(c):
        """k/v projections + feature map + causal scans for chunk c."""
        xk = xkp.tile([128, KT * CH], bf16, tag="xk")
        qsz = (KT // XQ) * CH
        # DMA emission order matters cold: first quarter, then head-0 k
        # weights, then the rest of xk — so head 0's matmuls start ASAP.
        nc.sync.dma_start(xk[:, 0:qsz], xk_d[c, :, 0:qsz])
        wkh0 = wtsp.tile([128, KT * 128], bf16, tag="wts")
        nc.sync.dma_start(wkh0[:], wk_d[0])
        for q in range(1, XQ):
            nc.sync.dma_start(xk[:, q * qsz:(q + 1) * qsz],
                              xk_d[c, :, q * qsz:(q + 1) * qsz])
        for h in range(HPC):
            if h == 0:
                wkh = wkh0
            else:
                wkh = wtsp.tile([128, KT * 128], bf16, tag="wts")
                nc.sync.dma_start(wkh[:], wk_d[h])
            pk = pp.tile([128, CH], fp32, tag="pp")
            proj_mm(pk, wkh, xk)
            e = tmpp.tile([128, CH], fp32, tag="tmp")
            nc.scalar.activation(e[:], pk[:], AF.Exp)
            r = tmpp.tile([128, CH], fp32, tag="tmp")
            nc.scalar.activation(r[:], pk[:], AF.Relu)
            kf = tmpp.tile([128, CH], fp32, tag="tmp")
            nc.vector.scalar_tensor_tensor(
                kf[:], e[:], 1.0, r[:], AL.min, AL.add)

            wvh = wtsp.tile([128, KT * 128], bf16, tag="wts")
            nc.sync.dma_start(wvh[:], wv_d[h])
            pv = pp.tile([128, CH], fp32, tag="pp")
            proj_mm(pv, wvh, xk)
            kv = tmpp.tile([128, CH], fp32, tag="tmp")
            # kv = kf * v, reading v straight from PSUM
            nc.vector.tensor_tensor(kv[:], kf[:], pv[:], AL.mult)

            kc = kcump.tile([128, CH], fp32, tag="kcum")
            init_k = 0.0 if c == 0 else kc_prev[h][:, CH - 1:CH]
            nc.vector.tensor_tensor_scan(
                kc[:], kf[:], kf[:], init_k, AL.add, AL.bypass)
            kvc = kvcump.tile([128, CH], fp32, tag="kvcum")
            init_kv = 0.0 if c == 0 else kvc_prev[h][:, CH - 1:CH]
            nc.vector.tensor_tensor_scan(
                kvc[:], kv[:], kv[:], init_kv, AL.add, AL.bypass)
            kc_cur[h] = kc
            kvc_cur[h] = kvc
        st[(c, "xk")] = xk

    def emit_B1(c):
        """q projection + feature map + z accumulation + w1 for chunk c."""
        pz = pzp.tile([128, CH], fp32, tag="pz")
        xk = st.pop((c, "xk"))
        w1s = []
        for h in range(HPC):
            wqh = wtsp.tile([128, KT * 128], bf16, tag="wts")
            nc.sync.dma_start(wqh[:], wq_d[h])
            pq = pp.tile([128, CH], fp32, tag="pp")
            proj_mm(pq, wqh, xk)
            eq = tmpp.tile([128, CH], fp32, tag="tmp")
            nc.scalar.activation(eq[:], pq[:], AF.Exp)
            rq = tmpp.tile([128, CH], fp32, tag="tmp")
            nc.scalar.activation(rq[:], pq[:], AF.Relu)
            qf = qfp.tile([128, CH], fp32, tag="qf")
            nc.vector.scalar_tensor_tensor(
                qf[:], eq[:], 1.0, rq[:], AL.min, AL.add)
            p = p32p.tile([128, CH], bf16, tag="p32r")
            with nc.allow_low_precision(reason="bf16 feeds FWL-rate PE z-reduce"):
                nc.vector.tensor_tensor(p[:], qf[:], kc_cur[h][:], AL.mult)
            nc.tensor.matmul(
                pz[:], consts["onehot"][:, h * 128:(h + 1) * 128], p[:],
                start=(h == 0), stop=(h == HPC - 1))
            w1s.append((qf, kvc_cur[h]))
        st[(c, "pz")] = pz
        st[(c, "w1")] = w1s

    def emit_B2(c, t0=0, t1=CH):
        """1/z + per-head broadcast + w tiles for chunk c, tokens [t0,t1)."""
        pz = st[(c, "pz")]
        w1s = st[(c, "w1")]
        if w1s and isinstance(w1s[0], tuple):
            mats = []
            for qf_t, kvc_t in w1s:
                w1 = w1p.tile([128, CH], bf16, tag="w1")
                nc.vector.tensor_tensor(w1[:], qf_t[:], kvc_t[:], AL.mult)
                mats.append(w1)
            w1s = st[(c, "w1")] = mats
        tw = t1 - t0
        zinv = smallp.tile([HPC, CH], bf16, tag="zinv")
        with nc.allow_low_precision(reason="bf16 feeds FWL-rate PE broadcast"):
            nc.vector.reciprocal(zinv[:, :tw], pz[0:HPC, t0:t1])
        whs = []
        for h in range(HPC):
            pzb = pzbp.tile([128, CH], fp32, tag="pzb")
            nc.tensor.matmul(pzb[:, :tw], consts["sel"][:, h * 128:(h + 1) * 128],
                             zinv[:, :tw], start=True, stop=True)
            wh = wtp.tile([128, CH], bf16, tag="wtile")
            nc.vector.tensor_tensor(wh[:, :tw], w1s[h][:, t0:t1], pzb[:, :tw],
                                    AL.mult)
            whs.append(wh)
        st[(c, "wh", t0)] = whs
        if t1 == CH:
            st.pop((c, "pz"))
            st.pop((c, "w1"))

    def emit_OUT(c, t0=0, t1=CH):
        """Row-sharded output projection partial for chunk c, tokens [t0,t1)."""
        whs = st.pop((c, "wh", t0))
        tw = t1 - t0
        for n in range(NT):
            won = wo_res[n]
            po = pop.tile([128, CH], fp32, tag="po")
            for hh in range(KO):
                nc.tensor.matmul(
                    po[:, :tw], won[:, hh * 128:(hh + 1) * 128],
                    whs[hh][:, :tw],
                    start=(hh == 0), stop=(hh == KO - 1))
            ot = outp.tile([128, CH], fp32, tag="outs")
            nc.scalar.copy(ot[:, :tw], po[:, :tw])
            nc.sync.dma_start(y_d[n, :, c * CH + t0:c * CH + t1], ot[:, :tw])

    wo_res = [None] * NT
    consts = {}
    for c in range(NCH):
        emit_A(c)
        if c == 0:
            # Constants + resident output-projection weights: emitted after
            # A(0) so the first chunk's xk/weight DMAs win queue priority.
            onehot = constp.tile([128, HPC * 128], bf16)
            nc.sync.dma_start(onehot[:], oh_d[:])
            sel = constp.tile([HPC, HPC * 128], bf16)
            nc.sync.dma_start(sel[:], sel_d[:])
            consts["onehot"] = onehot
            consts["sel"] = sel
            for n in range(NT):
                won = wop.tile([128, KO * 128], bf16, tag="wo")
                nc.sync.dma_start(won[:], wo_d[n])
                wo_res[n] = won
        if c > 0:
            emit_B2(c - 1)
        emit_B1(c)
        if c > 0:
            emit_OUT(c - 1)
        kc_prev = list(kc_cur)
        kvc_prev = list(kvc_cur)
    emit_B2(NCH - 1)
    emit_OUT(NCH - 1)


def _get_program():
    if "nc" not in _CACHE:
        _CACHE["nc"] = _build_program()
    return _CACHE["nc"]


def _prep_inputs(x, Wq, Wk, Wv, Wo):
    """Host-side shard + rearrange + cast. Returns per-core input maps."""
    def arrange_w_cols(W, g):
        # W[:, g*FPC:(g+1)*FPC] -> [HPC, 128, KT*128]
        Ws = np.ascontiguousarray(W[:, g * FPC:(g + 1) * FPC]).astype(BF16)
        return np.ascontiguousarray(
            Ws.reshape(KT, 128, HPC, 128).transpose(2, 1, 0, 3)
        ).reshape(HPC, 128, KT * 128)

    def arrange_wo_rows(W, g):
        # W[g*FPC:(g+1)*FPC, :] -> [NT, 128, KO*128]
        Ws = np.ascontiguousarray(W[g * FPC:(g + 1) * FPC, :]).astype(BF16)
        return np.ascontiguousarray(
            Ws.reshape(KO, 128, NT, 128).transpose(2, 1, 0, 3)
        ).reshape(NT, 128, KO * 128)

    onehot = np.zeros((128, HPC * 128), np.float32)
    for h in range(HPC):
        onehot[:, h * 128 + h] = 1.0
    onehot = onehot.astype(BF16)
    sel = np.zeros((HPC, HPC * 128), np.float32)
    for h in range(HPC):
        sel[h, h * 128:(h + 1) * 128] = 1.0
    sel = sel.astype(BF16)

    w_by_g = []
    for g in range(2):
        w_by_g.append({
            "wq": arrange_w_cols(Wq, g),
            "wk": arrange_w_cols(Wk, g),
            "wv": arrange_w_cols(Wv, g),
            "wo": arrange_wo_rows(Wo, g),
        })

    xk_by_b = []
    for b in range(B):
        xT = np.ascontiguousarray(x[b].T).astype(BF16)  # [D, L]
        xk = np.ascontiguousarray(
            xT.reshape(KT, 128, NCH, CH).transpose(2, 1, 0, 3)
        ).reshape(NCH, 128, KT * CH)
        xk_by_b.append(xk)

    in_maps = []
    for c in range(N_CORES):
        b, g = c // 2, c % 2
        m = {"xk": xk_by_b[b], "onehot": onehot, "sel": sel}
        m.update(w_by_g[g])
        in_maps.append(m)
    return in_maps


def _gather_output(results, bo):
    out = np.empty((B, L, D), np.float32)
    for b in range(B):
        yp = results[2 * b]["yT"] + results[2 * b + 1]["yT"]  # [NT,128,L]
        # yT[n, p, t] = out[t, n*128+p]
        out[b] = yp.reshape(NT * 128, L).T + bo[None, :]
    return out


def kernel(x, attention_mask, Wq, bq, Wk, bk, Wv, bv, Wo, bo, **_ignored):
    from concourse.bass_utils import run_bass_kernel_spmd

    x = np.asarray(x, np.float32)
    nc = _get_program()
    # bq/bk/bv are zero in this problem; q/k/v biases are additive constants
    # folded on host would be wrong (nonlinear feature map), so assert.
    assert not np.any(bq) and not np.any(bk) and not np.any(bv), \
        "kernel compiled for zero q/k/v biases"
    in_maps = _prep_inputs(x, np.asarray(Wq), np.asarray(Wk), np.asarray(Wv),
                           np.asarray(Wo))
    res = run_bass_kernel_spmd(nc, in_maps, list(range(N_CORES)))
    return _gather_output(res.results, np.asarray(bo, np.float32))


# revision 23
# speedup vs baseline: 1.0182x; 1.0042x over previous
"""Trainium2 Bass kernel for nn_PraxisAttention (causal linear attention).

Sharding: 8 cores = 4 batches x 2 head-groups (tensor-parallel over the 16
heads, per the sharding hint). Core c handles batch c//2 and heads
[8*(c%2), 8*(c%2)+8). Each core computes q/k/v projections for its 1024
feature columns (bf16 matmuls, fp32 accumulate), the elu(x)+1 feature map
(min(exp(x),1)+relu(x)), causal cumulative sums over the full 4096-token
sequence via DVE prefix scans, z = per-head dot(q, k_cum) via one-hot
reduction matmuls on the PE, and the row-sharded output projection, which
yields partial sums. The host adds the two partials per batch, re-adds bo,
and transposes back.

Pipelining: the per-chunk phases are emitted one chunk deep —
A(c) [k/v proj + scans], B2(c-1) [1/z + broadcast + w tiles], B1(c)
[q proj + z + w1], OUT(c-1) [output projection] — so the PE never stalls
on the DVE reciprocal / w-tile chain at chunk boundaries (which previously
triggered HAM re-throttling).

The attention_mask input is all-ones per the problem spec (a zero entry
would make the reference divide by zero), so multiplying k/v/z by it is an
identity and is skipped on device. EPS=1e-6 in the reference z denominator
is also skipped: all feature-map outputs are strictly positive so z >> EPS
(relative effect ~1e-7).

Numerics: matmul operands bf16 (fp32 PSUM accumulation); attention-core
intermediates fp32 except w1/w tiles (bf16); final partials stored fp32.
"""

import sys

sys.path.insert(0, "/opt/trn_rl_repo")

import numpy as np
import ml_dtypes

BF16 = ml_dtypes.bfloat16

# Problem constants
B, L, D = 4, 4096, 2048
H, DH = 16, 128
N_CORES = 8
HPC = 8        # heads per core
FPC = HPC * DH  # feature columns per core (1024)
CH = 512       # tokens per chunk
NCH = L // CH  # 8 chunks
KT = D // 128  # 16 k-tiles (projection contraction)
KO = FPC // 128  # 8 k-tiles (output projection contraction)
NT = D // 128  # 16 output feature tiles
XQ = 4         # xk DMA split (quarters)

_CACHE = {}


def _build_program(loop_r=None):
    """Build the per-core program. loop_r (timing only): wrap the whole body
    in a hardware For_i loop executing it loop_r times per dispatch."""
    import concourse.tile as tile
    from concourse import mybir, bacc

    fp32 = mybir.dt.float32
    bf16 = mybir.dt.bfloat16

    nc = bacc.Bacc("TRN2", target_bir_lowering=False, debug=False,
                   enable_asserts=True, num_devices=N_CORES)

    # Inputs (host pre-arranged, see kernel()):
    # xk[c][p][kk*CH+t] = x[b].T[kk*128+p, c*CH+t]
    xk_d = nc.dram_tensor("xk", [NCH, 128, KT * CH], bf16, kind="ExternalInput").ap()
    # wX[h][p][kk*128+j] = W[kk*128+p, h*128+j] (column-sharded slice)
    wq_d = nc.dram_tensor("wq", [HPC, 128, KT * 128], bf16, kind="ExternalInput").ap()
    wk_d = nc.dram_tensor("wk", [HPC, 128, KT * 128], bf16, kind="ExternalInput").ap()
    wv_d = nc.dram_tensor("wv", [HPC, 128, KT * 128], bf16, kind="ExternalInput").ap()
    # wo[n][p][hh*128+j] = Wo[rows][hh*128+p, n*128+j] (row-sharded slice)
    wo_d = nc.dram_tensor("wo", [NT, 128, KO * 128], bf16, kind="ExternalInput").ap()
    # onehot[:, h*8+m] = 1 iff m == h
    oh_d = nc.dram_tensor("onehot", [128, HPC * 128], bf16, kind="ExternalInput").ap()
    # sel[k, h*128+m] = 1 iff k == h  (broadcast row h of zinv over 128 partitions)
    sel_d = nc.dram_tensor("sel", [HPC, HPC * 128], bf16, kind="ExternalInput").ap()
    # Output: partial yT[n][p][c*CH+t] = sum over this core's features
    y_d = nc.dram_tensor("yT", [NT, 128, L], fp32, kind="ExternalOutput").ap()

    with tile.TileContext(nc) as tc:
        with (
            tc.tile_pool(name="const", bufs=1) as constp,
            tc.tile_pool(name="xk", bufs=2) as xkp,
            tc.tile_pool(name="wts", bufs=6) as wtsp,
            tc.tile_pool(name="wo", bufs=16) as wop,
            tc.tile_pool(name="tmp", bufs=8) as tmpp,
            tc.tile_pool(name="p32", bufs=4) as p32p,
            tc.tile_pool(name="kcum", bufs=10) as kcump,
            tc.tile_pool(name="kvcum", bufs=9) as kvcump,
            tc.tile_pool(name="qf", bufs=10) as qfp,
            tc.tile_pool(name="w1", bufs=10) as w1p,
            tc.tile_pool(name="wtile", bufs=16) as wtp,
            tc.tile_pool(name="small", bufs=2) as smallp,
            tc.tile_pool(name="outs", bufs=4) as outp,
            tc.tile_pool(name="pp", bufs=3, space="PSUM") as pp,
            tc.tile_pool(name="pz", bufs=1, space="PSUM") as pzp,
            tc.tile_pool(name="pzb", bufs=2, space="PSUM") as pzbp,
            tc.tile_pool(name="po", bufs=2, space="PSUM") as pop,
        ):
            import contextlib
            loop_ctx = (tc.For_i(0, loop_r, 1) if loop_r
                        else contextlib.nullcontext())
            with loop_ctx:
                _body(nc, tc, mybir, xk_d, wq_d, wk_d, wv_d, wo_d, y_d,
                      oh_d, sel_d, constp,
                      xkp, wtsp, wop, tmpp, p32p, kcump, kvcump, qfp, w1p, wtp,
                      smallp, outp, pp, pzp, pzbp, pop)

    nc.compile()
    return nc


def _body(nc, tc, mybir, xk_d, wq_d, wk_d, wv_d, wo_d, y_d,
          oh_d, sel_d, constp,
          xkp, wtsp, wop, tmpp, p32p, kcump, kvcump, qfp, w1p, wtp,
          smallp, outp, pp, pzp, pzbp, pop):
    fp32 = mybir.dt.float32
    bf16 = mybir.dt.bfloat16
    f32r = mybir.dt.float32r
    AL = mybir.AluOpType
    AF = mybir.ActivationFunctionType

    # Per-chunk state carried between phases / chunks.
    kc_prev = [None] * HPC    # k-cumsum tiles of previous chunk
    kvc_prev = [None] * HPC   # kv-cumsum tiles of previous chunk
    kc_cur = [None] * HPC
    kvc_cur = [None] * HPC
    st = {}                   # per-chunk: pz tile, w1 tiles, wh tiles

    def proj_mm(ptile, wtile, xk):
        for kk in range(KT):
            nc.tensor.matmul(
                ptile[:], wtile[:, kk * 128:(kk + 1) * 128],
                xk[:, kk * CH:(kk + 1) * CH],
                start=(kk == 0), stop=(kk == KT - 1))

    def emit_A(c):
        """k/v projections + feature map + causal scans for chunk c."""
        xk = xkp.tile([128, KT * CH], bf16, tag="xk")
        qsz = (KT // XQ) * CH
        # DMA emission order matters cold: first quarter, then head-0 k
        # weights, then the rest of xk — so head 0's matmuls start ASAP.
        nc.sync.dma_start(xk[:, 0:qsz], xk_d[c, :, 0:qsz])
        wkh0 = wtsp.tile([128, KT * 128], bf16, tag="wts")
        nc.sync.dma_start(wkh0[:], wk_d[0])
        for q in range(1, XQ):
            nc.sync.dma_start(xk[:, q * qsz:(q + 1) * qsz],
                              xk_d[c, :, q * qsz:(q + 1) * qsz])
        for h in range(HPC):
            if h == 0:
                wkh = wkh0
            else:
                wkh = wtsp.tile([128, KT * 128], bf16, tag="wts")
                nc.sync.dma_start(wkh[:], wk_d[h])
            pk = pp.tile([128, CH], fp32, tag="pp")
            proj_mm(pk, wkh, xk)
            e = tmpp.tile([128, CH], fp32, tag="tmp")
            nc.scalar.activation(e[:], pk[:], AF.Exp)
            r = tmpp.tile([128, CH], fp32, tag="tmp")
            nc.scalar.activation(r[:], pk[:], AF.Relu)
            kf = tmpp.tile([128, CH], fp32, tag="tmp")
            nc.vector.scalar_tensor_tensor(
                kf[:], e[:], 1.0, r[:], AL.min, AL.add)

            wvh = wtsp.tile([128, KT * 128], bf16, tag="wts")
            nc.sync.dma_start(wvh[:], wv_d[h])
            pv = pp.tile([128, CH], fp32, tag="pp")
            proj_mm(pv, wvh, xk)
            kv = tmpp.tile([128, CH], fp32, tag="tmp")
            # kv = kf * v, reading v straight from PSUM
            nc.vector.tensor_tensor(kv[:], kf[:], pv[:], AL.mult)

            kc = kcump.tile([128, CH], fp32, tag="kcum")
            init_k = 0.0 if c == 0 else kc_prev[h][:, CH - 1:CH]
            nc.vector.tensor_tensor_scan(
                kc[:], kf[:], kf[:], init_k, AL.add, AL.bypass)
            kvc = kvcump.tile([128, CH], fp32, tag="kvcum")
            init_kv = 0.0 if c == 0 else kvc_prev[h][:, CH - 1:CH]
            nc.vector.tensor_tensor_scan(
                kvc[:], kv[:], kv[:], init_kv, AL.add, AL.bypass)
            kc_cur[h] = kc
            kvc_cur[h] = kvc
        st[(c, "xk")] = xk

    def emit_B1(c):
        """q projection + feature map + z accumulation + w1 for chunk c."""
        pz = pzp.tile([128, CH], fp32, tag="pz")
        pending_selwh = st.pop((c - 1, "selwh"), []) if c > 0 else []
        xk = st.pop((c, "xk"))
        w1s = []
        for h in range(HPC):
            wqh = wtsp.tile([128, KT * 128], bf16, tag="wts")
            nc.sync.dma_start(wqh[:], wq_d[h])
            pq = pp.tile([128, CH], fp32, tag="pp")
            proj_mm(pq, wqh, xk)
            eq = tmpp.tile([128, CH], fp32, tag="tmp")
            nc.scalar.activation(eq[:], pq[:], AF.Exp)
            rq = tmpp.tile([128, CH], fp32, tag="tmp")
            nc.scalar.activation(rq[:], pq[:], AF.Relu)
            qf = qfp.tile([128, CH], fp32, tag="qf")
            nc.vector.scalar_tensor_tensor(
                qf[:], eq[:], 1.0, rq[:], AL.min, AL.add)
            p = p32p.tile([128, CH], bf16, tag="p32r")
            with nc.allow_low_precision(reason="bf16 feeds FWL-rate PE z-reduce"):
                nc.vector.tensor_tensor(p[:], qf[:], kc_cur[h][:], AL.mult)
            nc.tensor.matmul(
                pz[:], consts["onehot"][:, h * 128:(h + 1) * 128], p[:],
                start=(h == 0), stop=(h == HPC - 1))
            if pending_selwh:
                pending_selwh.pop(0)()
            w1s.append((qf, kvc_cur[h]))
        st[(c, "pz")] = pz
        st[(c, "w1")] = w1s

    def emit_B2(c, t0=0, t1=CH):
        """1/z + per-head broadcast + w tiles for chunk c, tokens [t0,t1)."""
        pz = st[(c, "pz")]
        w1s = st[(c, "w1")]
        if w1s and isinstance(w1s[0], tuple):
            mats = []
            for qf_t, kvc_t in w1s:
                w1 = w1p.tile([128, CH], bf16, tag="w1")
                nc.vector.tensor_tensor(w1[:], qf_t[:], kvc_t[:], AL.mult)
                mats.append(w1)
            w1s = st[(c, "w1")] = mats
        tw = t1 - t0
        zinv = smallp.tile([HPC, CH], bf16, tag="zinv")
        with nc.allow_low_precision(reason="bf16 feeds FWL-rate PE broadcast"):
            nc.vector.reciprocal(zinv[:, :tw], pz[0:HPC, t0:t1])
        whs = []

        def mk_selwh(h, _zinv=zinv, _w1s=w1s, _whs=whs, _tw=tw, _t0=t0, _t1=t1):
            pzb = pzbp.tile([128, CH], fp32, tag="pzb")
            nc.tensor.matmul(pzb[:, :_tw], consts["sel"][:, h * 128:(h + 1) * 128],
                             _zinv[:, :_tw], start=True, stop=True)
            wh = wtp.tile([128, CH], bf16, tag="wtile")
            nc.vector.tensor_tensor(wh[:, :_tw], _w1s[h][:, _t0:_t1],
                                    pzb[:, :_tw], AL.mult)
            _whs.append(wh)

        st[(c, "selwh")] = [lambda h=h: mk_selwh(h) for h in range(HPC)]
        st[(c, "wh", t0)] = whs
        if t1 == CH:
            st.pop((c, "pz"))
            st.pop((c, "w1"))

    def emit_OUT(c, t0=0, t1=CH):
        """Row-sharded output projection partial for chunk c, tokens [t0,t1)."""
        whs = st.pop((c, "wh", t0))
        tw = t1 - t0
        for n in range(NT):
            won = wo_res[n]
            po = pop.tile([128, CH], fp32, tag="po")
            for hh in range(KO):
                nc.tensor.matmul(
                    po[:, :tw], won[:, hh * 128:(hh + 1) * 128],
                    whs[hh][:, :tw],
                    start=(hh == 0), stop=(hh == KO - 1))
            ot = outp.tile([128, CH], fp32, tag="outs")
            nc.scalar.copy(ot[:, :tw], po[:, :tw])
            nc.sync.dma_start(y_d[n, :, c * CH + t0:c * CH + t1], ot[:, :tw])

    wo_res = [None] * NT
    consts = {}
    for c in range(NCH):
        emit_A(c)
        if c == 0:
            # Constants + resident output-projection weights: emitted after
            # A(0) so the first chunk's xk/weight DMAs win queue priority.
            onehot = constp.tile([128, HPC * 128], bf16)
            nc.sync.dma_start(onehot[:], oh_d[:])
            sel = constp.tile([HPC, HPC * 128], bf16)
            nc.sync.dma_start(sel[:], sel_d[:])
            consts["onehot"] = onehot
            consts["sel"] = sel
            for n in range(NT):
                won = wop.tile([128, KO * 128], bf16, tag="wo")
                nc.sync.dma_start(won[:], wo_d[n])
                wo_res[n] = won
        if c > 0:
            emit_B2(c - 1)
        emit_B1(c)
        if c > 0:
            emit_OUT(c - 1)
        kc_prev = list(kc_cur)
        kvc_prev = list(kvc_cur)
    emit_B2(NCH - 1)
    for fn in st.pop((NCH - 1, "selwh")):
        fn()
    emit_OUT(NCH - 1)


def _get_program():
    if "nc" not in _CACHE:
        _CACHE["nc"] = _build_program()
    return _CACHE["nc"]


def _prep_inputs(x, Wq, Wk, Wv, Wo):
    """Host-side shard + rearrange + cast. Returns per-core input maps."""
    def arrange_w_cols(W, g):
        # W[:, g*FPC:(g+1)*FPC] -> [HPC, 128, KT*128]
        Ws = np.ascontiguousarray(W[:, g * FPC:(g + 1) * FPC]).astype(BF16)
        return np.ascontiguousarray(
            Ws.reshape(KT, 128, HPC, 128).transpose(2, 1, 0, 3)
        ).reshape(HPC, 128, KT * 128)

    def arrange_wo_rows(W, g):
        # W[g*FPC:(g+1)*FPC, :] -> [NT, 128, KO*128]
        Ws = np.ascontiguousarray(W[g * FPC:(g + 1) * FPC, :]).astype(BF16)
        return np.ascontiguousarray(
            Ws.reshape(KO, 128, NT, 128).transpose(2, 1, 0, 3)
        ).reshape(NT, 128, KO * 128)

    onehot = np.zeros((128, HPC * 128), np.float32)
    for h in range(HPC):
        onehot[:, h * 128 + h] = 1.0
    onehot = onehot.astype(BF16)
    sel = np.zeros((HPC, HPC * 128), np.float32)
    for h in range(HPC):
        sel[h, h * 128:(h + 1) * 128] = 1.0
    sel = sel.astype(BF16)

    w_by_g = []
    for g in range(2):
        w_by_g.append({
            "wq": arrange_w_cols(Wq, g),
            "wk": arrange_w_cols(Wk, g),
            "wv": arrange_w_cols(Wv, g),
            "wo": arrange_wo_rows(Wo, g),
        })

    xk_by_b = []
    for b in range(B):
        xT = np.ascontiguousarray(x[b].T).astype(BF16)  # [D, L]
        xk = np.ascontiguousarray(
            xT.reshape(KT, 128, NCH, CH).transpose(2, 1, 0, 3)
        ).reshape(NCH, 128, KT * CH)
        xk_by_b.append(xk)

    in_maps = []
    for c in range(N_CORES):
        b, g = c // 2, c % 2
        m = {"xk": xk_by_b[b], "onehot": onehot, "sel": sel}
        m.update(w_by_g[g])
        in_maps.append(m)
    return in_maps


def _gather_output(results, bo):
    out = np.empty((B, L, D), np.float32)
    for b in range(B):
        yp = results[2 * b]["yT"] + results[2 * b + 1]["yT"]  # [NT,128,L]
        # yT[n, p, t] = out[t, n*128+p]
        out[b] = yp.reshape(NT * 128, L).T + bo[None, :]
    return out


def kernel(x, attention_mask, Wq, bq, Wk, bk, Wv, bv, Wo, bo, **_ignored):
    from concourse.bass_utils import run_bass_kernel_spmd

    x = np.asarray(x, np.float32)
    nc = _get_program()
    # bq/bk/bv are zero in this problem; q/k/v biases are additive constants
    # folded on host would be wrong (nonlinear feature map), so assert.
    assert not np.any(bq) and not np.any(bk) and not np.any(bv), \
        "kernel compiled for zero q/k/v biases"
    in_maps = _prep_inputs(x, np.asarray(Wq), np.asarray(Wk), np.asarray(Wv),
                           np.asarray(Wo))
    res = run_bass_kernel_spmd(nc, in_maps, list(range(N_CORES)))
    return _gather_output(res.results, np.asarray(bo, np.float32))
